# revision 36
# baseline (speedup 1.0000x reference)
"""Multi-head attention (QKV projection + masked softmax + PV) on 8 TRN2
NeuronCores.

Sharding: data-parallel over batch (B=2 -> 2 groups of 4 cores), tensor
parallel over heads (16 heads -> 4 heads per core). Each core computes full
F x T attention for its 4 heads.

Schedule (per core), built from HW-microbenchmarked facts:
  - exp on ScalarE only (16.8M elements at ~1 elem/lane/cycle is a ~133us
    floor -- the kernel bottleneck); everything else is kept off ScalarE,
    and in the steady state the exp stream runs back-to-back (~1005ns per
    [128, 2, 512] psum pair) with the PE ~86% subscribed underneath it.
  - every matmul runs in the same 128x128 PE tiling mode: the S stationaries
    are per-head ZERO-PADDED to K=128 (KT slot 0 = even head in rows 0-63,
    slot 1 = odd head in rows 64-127, dead halves zero). Row-tiled 64-row
    S pairs would be 2x faster in isolation, but with only two S psum
    buffers the scheduler must interleave S with PV every t-step and each
    (64,128)<->(128,128) mode switch drains the array (~110ns measured), so
    uniform-mode zero-padded S is net equal and HAM-stable.
  - PV runs one full (j, g)-phase behind S with the whole phase's masked-exp
    tiles buffered in SBUF: j0's projection pieces (all V/K tiles) then
    spread over two phases, and every later phase absorbs the previous
    pair's PV into its ACT-bound slack.
  - mask multiply fused over both heads in ONE DVE op using a 0-stride
    broadcast AP on the mask (691ns per [128, 2, 512] at DVE 2x).
  - softmax sums via the leading ones column in the V stationary (M=65 PV);
    normalization = psum evac + reciprocal_approx_fast + f16 shadow copy
    (all DVE) -> sel64 broadcast matmul (PE) -> fused norm multiply (DVE)
    -> DMA out, with the matmul/multiply deferred into later phases'
    t-slots so the in-order PE queue never waits on the DVE chain.
    (GpSimd ISA ops looked ideal for this but measured catastrophically
    slow end-to-end -- library reload serialization; avoid.)
  - bulk loads ride the HW-DGE (sync) queue in consumer-deadline order
    (per-k chunks so range deps release consumers early; each dma_start
    costs ~1us of queue issue, and batched loads pay issue per descriptor
    so they are no faster); wq/fromT ride the gpsimd SW-DGE queue in
    parallel; output stores ride sync behind all input loads.
  - ~120 short dummy matmuls + one Exp warm the PE clock gate (HAM) and
    the ACT table and keep the clock hot across the input-DMA window, so
    the first K/Q/S pieces run at 2.4GHz instead of 1.2; the first four V
    tiles are computed in the prologue (interleaved around Q) to convert
    the fromT-arrival wait into real work and unload phase (0,0).
"""

import os
import sys

for _p in ("/opt/trn_rl_repo",):
    if os.path.isdir(_p) and _p not in sys.path:
        sys.path.insert(0, _p)

import numpy as np
import ml_dtypes

import concourse.tile as tile
from concourse import bacc, mybir
from concourse.bass import broadcast_tensor_aps
from concourse.bass_utils import run_bass_kernel_spmd

B, F, T, D, N, H = 2, 2048, 2048, 1024, 16, 64
NCORES = 8
HPC = N // (NCORES // B)  # heads per core = 4
NG = HPC // 2             # 128-partition head pairs = 2
FB = 512                  # f-block (psum bank width in fp32)
NJ = F // FB              # 4
NT = T // 128             # 16 t-tiles
NK = D // 128             # 8 contraction tiles
HP1 = H + 1               # head V columns incl. the ones column
LAG = 2                   # PV runs LAG t-tiles behind S/exp
NBLK = NT // 2            # 2-t-step blocks per (j, g)

F32 = mybir.dt.float32
F16 = mybir.dt.float16
BF16 = mybir.dt.bfloat16


def _emit_k_piece(nc, ps1, kv, KT, bias_sb, tb, g):
    """KT[:, g, i, tb*FB:(tb+1)*FB] for head pair g. KT keeps one K=128
    zero-padded stationary per head (even head in rows 0-63 of slot 0, odd
    head in rows 64-127 of slot 1, complementary halves stay zero) so the S
    matmuls run full-contraction in the same PE tiling mode as everything
    else -- no mode-switch drains. Bias adds ride the DVE evacuation."""
    toT_sb, wk_sb = kv["toT_sb"], kv["wk_sb"]
    ps_qk = ps1.tile([128, FB], F32, tag="p1", name="ps_k")
    for k in range(NK):
        nc.tensor.matmul(
            ps_qk[:],
            wk_sb[:, k, g * 128:(g + 1) * 128],
            toT_sb[:, k, tb * FB:(tb + 1) * FB],
            start=(k == 0),
            stop=(k == NK - 1),
        )
    nc.vector.tensor_scalar_add(
        KT[0:64, g, 0, tb * FB:(tb + 1) * FB],
        ps_qk[0:64, :],
        bias_sb[0:64, NG + g:NG + g + 1],
    )
    nc.vector.tensor_scalar_add(
        KT[64:128, g, 1, tb * FB:(tb + 1) * FB],
        ps_qk[64:128, :],
        bias_sb[64:128, NG + g:NG + g + 1],
    )


def _emit_v_piece(nc, ps1, kv, Vsb, bv_sb, vones_sb, ti):
    """V[t-tile ti] for all 4 heads, interleaved with the ones columns."""
    toT_sb, wv_sb = kv["toT_sb"], kv["wv_sb"]
    ps_v = ps1.tile([128, FB], F32, tag="p1", name="ps_v")
    for k in range(NK):
        nc.tensor.matmul(
            ps_v[:, 0:HPC * H],
            toT_sb[:, k, ti * 128:(ti + 1) * 128],
            wv_sb[:, k, :],
            start=(k == 0),
            stop=False,
        )
    nc.tensor.matmul(ps_v[:, 0:HPC * H], vones_sb[:], bv_sb[:],
                     start=False, stop=True)
    # one strided cast per t-tile: [128, 4, 64] -> [128, 4, 65][:, :, 1:65]
    dst = Vsb[:, ti, :].rearrange("p (n h1) -> p n h1", n=HPC)
    src = ps_v[:, 0:HPC * H].rearrange("p (n h) -> p n h", n=HPC)
    nc.vector.tensor_copy(dst[:, :, 1:HP1], src)


def _emit_qt(nc, ps1, wq_sb, fromT_tile, QT, bias_sb, j, g):
    ps_qk = ps1.tile([128, FB], F32, tag="p1", name="ps_q")
    for k in range(NK):
        nc.tensor.matmul(
            ps_qk[:],
            wq_sb[:, k, g * 128:(g + 1) * 128],
            fromT_tile[:, k, :],
            start=(k == 0),
            stop=(k == NK - 1),
        )
    nc.vector.tensor_scalar_add(
        QT[:, g, j * FB:(j + 1) * FB],
        ps_qk[:],
        bias_sb[:, g:g + 1],
    )


def _program():
    nc = bacc.Bacc(None, target_bir_lowering=False)
    fromT = nc.declare_dram_parameter("fromT", [D, F], BF16, isOutput=False)
    toT = nc.declare_dram_parameter("toT", [D, T], BF16, isOutput=False)
    maskT = nc.declare_dram_parameter("maskT", [T, F], BF16, isOutput=False)
    wq = nc.declare_dram_parameter("wq", [D, HPC * H], BF16, isOutput=False)
    wk = nc.declare_dram_parameter("wk", [D, HPC * H], BF16, isOutput=False)
    wv = nc.declare_dram_parameter("wv", [D, HPC * H], BF16, isOutput=False)
    bqk = nc.declare_dram_parameter("bqk", [128, 2 * NG], F32, isOutput=False)
    # bv padded to K=128 (row 0 = bv, rest zero) for a mode-switch-free matmul
    bv_pad = nc.declare_dram_parameter("bv_pad", [128, HPC * H], BF16, isOutput=False)
    # all-ones row 0 (rest zero): stationary operand of the bv matmul
    vones = nc.declare_dram_parameter("vones", [128, 128], BF16, isOutput=False)
    # broadcast selector: sel64[k, m] = (k == 0); as lhsT it replicates the
    # reciprocal row (partition 0 of rsh) down all 128 output partitions
    sel64 = nc.declare_dram_parameter("sel64", [128, 128], F16, isOutput=False)
    out_ctx = nc.declare_dram_parameter("out_ctx", [HPC, H, F], F32, isOutput=True)

    fromT_re = fromT[:].rearrange("(k p) f -> p k f", p=128)
    toT_re = toT[:].rearrange("(k p) t -> p k t", p=128)
    maskT_re = maskT[:].rearrange("(a p) f -> p a f", p=128)

    with tile.TileContext(nc) as tc:
        with (
            tc.tile_pool(name="persist", bufs=1) as persist,
            tc.tile_pool(name="p1", bufs=1) as p1,
            tc.tile_pool(name="pfrom", bufs=2) as pfrom,
            tc.tile_pool(name="pmask", bufs=2) as pmask,
            tc.tile_pool(name="pex", bufs=NT + 3) as pex,
            tc.tile_pool(name="pctx", bufs=2) as pctx,
            tc.tile_pool(name="pout", bufs=4) as pout,
            tc.tile_pool(name="prcp", bufs=4) as prcp,
            tc.tile_pool(name="pbc", bufs=4) as pbc,
            tc.tile_pool(name="ps1", bufs=2, space="PSUM") as ps1,
            tc.tile_pool(name="ps_s", bufs=2, space="PSUM") as ps_s,
            tc.tile_pool(name="ps_c", bufs=1, space="PSUM") as ps_c,
        ):
            QT = persist.tile([128, NG, F], BF16)   # [h-in-pair, g, f]
            # per-head zero-padded K=128 stationaries: slot 0 = even head
            # (rows 0-63 live), slot 1 = odd head (rows 64-127 live)
            KT = persist.tile([128, NG, 2, T], BF16)
            Vsb = persist.tile([128, NT, HPC * HP1], BF16)
            bias_sb = persist.tile([128, 2 * NG], F32)
            bv_sb = persist.tile([128, HPC * H], BF16)
            vones_sb = persist.tile([128, 128], BF16)
            sel64_sb = persist.tile([128, 128], F16)
            # recip shadow: partition 0 of rsh[:, g, i, :] holds 1/sums in
            # f16; rows 1-64 stay zero so the selector matmul is clean
            rsh = persist.tile([HP1, NG, 2, FB], F16)
            warm_w = persist.tile([128, 128], BF16)
            warm_m = persist.tile([128, FB], BF16)

            # ---- t=0: warm the ACT table + the PE clock gate (no DMA deps)
            nc.vector.memset(warm_w[:], 0.0)
            nc.vector.memset(warm_m[:], 0.0)
            nc.vector.memset(rsh[:], 0.0)
            # zero the dead halves of the per-head K stationaries once
            nc.vector.memset(KT[64:128, :, 0, :], 0.0)
            nc.vector.memset(KT[0:64, :, 1, :], 0.0)
            for nl in range(HPC):
                nc.vector.memset(Vsb[:, :, nl * HP1], 1.0)
            act_warm = persist.tile([1, 1], F32)
            nc.scalar.activation(act_warm[:], warm_m[0:1, 0:1],
                                 mybir.ActivationFunctionType.Exp)
            ps_warm_a = ps1.tile([128, FB], F32, tag="p1", name="ps_warm")
            ps_warm_b = ps1.tile([128, FB], F32, tag="p1", name="ps_warm")
            # many SHORT warm matmuls: keeps the PE HAM clock-gate hot across
            # the whole input-DMA window (~8-16us) so the first K/Q/S pieces
            # run at 2.4GHz, while each tiny mm adds at most ~56ns of queue
            # delay ahead of the real work
            for i in range(120):
                nc.tensor.matmul((ps_warm_a if i % 2 == 0 else ps_warm_b)[:, 0:128],
                                 warm_w[:], warm_m[:, 0:128],
                                 start=True, stop=True)

            # ---- DMA issue. Everything on the j0 critical path goes on the
            # HW-DGE (sync) queue in deadline order -- the SW-DGE (gpsimd)
            # path only sustains ~1/3 the bandwidth. gpsimd carries the
            # late-deadline fromT prefetches.
            toT_sb = p1.tile([128, NK, T], BF16)
            wq_sb = p1.tile([128, NK, HPC * H], BF16)
            wk_sb = p1.tile([128, NK, HPC * H], BF16)
            wv_sb = p1.tile([128, NK, HPC * H], BF16)
            nc.sync.dma_start(bias_sb[:], bqk[:])
            for k in range(NK):
                nc.sync.dma_start(wk_sb[:, k, :], wk[k * 128:(k + 1) * 128, :])
                nc.sync.dma_start(toT_sb[:, k, 0:FB], toT_re[:, k, 0:FB])
            fromT_t = {}
            fromT_t[0] = pfrom.tile([128, NK, FB], BF16, tag="fromT", name="fromT")
            for k in range(NK):
                nc.gpsimd.dma_start(wq_sb[:, k, :], wq[k * 128:(k + 1) * 128, :])
                nc.gpsimd.dma_start(fromT_t[0][:, k, :], fromT_re[:, k, 0:FB])
            nc.gpsimd.dma_start(bv_sb[:], bv_pad[:])
            nc.gpsimd.dma_start(vones_sb[:], vones[:])
            nc.gpsimd.dma_start(sel64_sb[:], sel64[:])
            masks = {}
            masks[0] = pmask.tile([128, NT, FB], BF16, tag="mask", name="mask")
            nc.sync.dma_start(wv_sb[:], wv[:].rearrange("(k p) m -> p k m", p=128))
            nc.sync.dma_start(masks[0][:, 0:NT // 2, :],
                              maskT_re[:, 0:NT // 2, 0:FB])
            nc.sync.dma_start(
                toT_sb[:, :, FB:2 * FB], toT_re[:, :, FB:2 * FB])
            nc.sync.dma_start(
                toT_sb[:, :, 2 * FB:3 * FB], toT_re[:, :, 2 * FB:3 * FB])
            nc.sync.dma_start(masks[0][:, NT // 2:NT, :],
                              maskT_re[:, NT // 2:NT, 0:FB])
            nc.sync.dma_start(
                toT_sb[:, :, 3 * FB:4 * FB], toT_re[:, :, 3 * FB:4 * FB])
            fromT_t[1] = pfrom.tile([128, NK, FB], BF16, tag="fromT", name="fromT")
            nc.gpsimd.dma_start(fromT_t[1][:], fromT_re[:, :, FB:2 * FB])

            kv = dict(toT_sb=toT_sb, wk_sb=wk_sb, wv_sb=wv_sb)

            # ---- piece schedule: (j, g) -> {block: [piece...]} ------------
            # deadlines: V(ti) before PV consumes it (block ti//2 + 1);
            # K(tb, g') before S(g', 4tb); QT(j') before (j', g') starts.
            def V(ti):
                return lambda: _emit_v_piece(nc, ps1, kv, Vsb, bv_sb,
                                             vones_sb, ti)

            def K(tb, g):
                return lambda: _emit_k_piece(nc, ps1, kv, KT, bias_sb, tb, g)

            def Q(j, g):
                return lambda: _emit_qt(nc, ps1, wq_sb, fromT_t[j], QT,
                                        bias_sb, j, g)

            # PV runs one full (j, g)-phase behind S (the whole phase's mexp
            # is buffered in SBUF), so j0's projection pieces spread over two
            # phases and every later phase inherits the previous pair's PV
            # into its ACT-bound slack.
            SCHED = {
                (0, 0): {0: [V(4)], 1: [V(5), K(1, 0)], 2: [V(6)],
                         3: [V(7), K(2, 0)], 5: [K(3, 0)],
                         6: [K(0, 1)], 8: [Q(0, 1)]},
                (0, 1): {0: [V(8), K(1, 1)], 1: [V(9)], 2: [V(10), K(2, 1)],
                         3: [V(11)], 4: [V(12), K(3, 1)],
                         5: [V(13), Q(1, 0)], 6: [V(14)], 7: [V(15)]},
                (1, 0): {5: [Q(1, 1)]},
                (1, 1): {5: [Q(2, 0)]},
                (2, 0): {5: [Q(2, 1)]},
                (2, 1): {5: [Q(3, 0)]},
                (3, 0): {5: [Q(3, 1)]},
            }

            # ---- prologue: K first, then the first four V tiles (their
            # inputs land ~15us while Q's fromT rides the slow SW-DGE path
            # until ~27us -- real work replaces dummy warm matmuls in the
            # startup window), then Q
            _emit_k_piece(nc, ps1, kv, KT, bias_sb, 0, 0)
            _emit_v_piece(nc, ps1, kv, Vsb, bv_sb, vones_sb, 0)
            _emit_v_piece(nc, ps1, kv, Vsb, bv_sb, vones_sb, 1)
            _emit_qt(nc, ps1, wq_sb, fromT_t[0], QT, bias_sb, 0, 0)
            _emit_v_piece(nc, ps1, kv, Vsb, bv_sb, vones_sb, 2)
            _emit_v_piece(nc, ps1, kv, Vsb, bv_sb, vones_sb, 3)

            # ---- attention ----
            norm_q = []           # deferred per-head norm closures
            mul_q = []            # their pending multiply+store halves

            def _norm_pair(j, g, ctx_sb):
                """Queue the two per-head norm emitters for (j, g), split
                into a bc-matmul half and a multiply+store half so a single
                t-slot never absorbs both the PE and the DVE overhead."""
                for i in range(2):
                    nn = 2 * g + i
                    box = []

                    def _bc(i=i, g=g, box=box):
                        ps_bc = ps1.tile([128, FB], F32, tag="p1", name="ps_bc")
                        nc.tensor.matmul(
                            ps_bc[:],
                            sel64_sb[0:HP1, :],
                            rsh[:, g, i, :],
                            start=True, stop=True,
                        )
                        box.append(ps_bc)

                    def _mul(i=i, nn=nn, j=j, ctx_sb=ctx_sb, box=box):
                        ps_bc = box.pop()
                        out_f = pout.tile([HP1, FB], F32, tag="out", name="out")
                        nc.vector.tensor_mul(
                            out_f[:], ctx_sb[:, i, :], ps_bc[0:HP1, :])
                        nc.sync.dma_start(
                            out_ctx[nn, :, j * FB:(j + 1) * FB],
                            out_f[1:HP1, :])
                    norm_q.append((_bc, _mul))

            phases = [(j, g) for j in range(NJ) for g in range(NG)]
            prev_pair = None      # pair whose PV runs in the current phase
            prev_exs = None       # that pair's mexp tiles, one per t-tile
            for pair in phases + [None]:
                if pair is not None and pair[1] == 0:
                    j = pair[0]
                    if j + 1 < NJ:
                        masks[j + 1] = pmask.tile([128, NT, FB], BF16,
                                                  tag="mask", name="mask")
                        nc.sync.dma_start(
                            masks[j + 1][:],
                            maskT_re[:, :, (j + 1) * FB:(j + 2) * FB],
                        )
                    if j + 2 < NJ:
                        fromT_t[j + 2] = pfrom.tile([128, NK, FB], BF16,
                                                    tag="fromT", name="fromT")
                        nc.gpsimd.dma_start(
                            fromT_t[j + 2][:],
                            fromT_re[:, :, (j + 2) * FB:(j + 3) * FB],
                        )
                sched = SCHED.get(pair, {}) if pair is not None else {}
                if pair is not None:
                    j, g = pair
                    mask_j = masks[j]
                if prev_pair is not None:
                    pj, pg = prev_pair
                    pne, pno = 2 * pg, 2 * pg + 1
                    ps_ctx = ps_c.tile([HP1, 2, FB], F32, tag="ctx",
                                       name="ctx")
                if pair is None:
                    # drain phase: the first PV chains wait ~1.5us for the
                    # previous pair's psum evacuation; without PE activity the
                    # HAM clock-gate re-throttles and all 32 drain matmuls
                    # run at 1.2GHz. Bridge the wait with dependency-free
                    # warm matmuls so the drain runs at full clock.
                    wt = ps_s.tile([128, 2, FB], F32, tag="sq", name="sq")
                    for i in range(16):
                        nc.tensor.matmul(wt[:, 0, 0:128], warm_w[:],
                                         warm_m[:, 0:128],
                                         start=True, stop=True)
                exs = {}
                for t in range(NT + 1):
                    # deferred norm + projection pieces first so the in-order
                    # PE queue has ready work ahead of exp-blocked S pairs
                    if t in (5, 9) and norm_q:
                        item = norm_q.pop(0)
                        item[0]()
                        mul_q.append(item[1])
                    if t in (8, 12) and mul_q:
                        mul_q.pop(0)()
                    if t % 2 == 0 and t // 2 in sched:
                        for piece in sched[t // 2]:
                            piece()
                    # S first each step so exp fires as early as possible
                    # (the scheduler honors program order among ready PE ops;
                    # exp paces the whole pipeline)
                    if pair is not None and t < NT:
                        ps_sq = ps_s.tile([128, 2, FB], F32,
                                          tag="sq", name="sq")
                        nc.tensor.matmul(
                            ps_sq[:, 0, :],
                            KT[:, g, 0, t * 128:(t + 1) * 128],
                            QT[:, g, j * FB:(j + 1) * FB],
                            start=True, stop=True,
                        )
                        nc.tensor.matmul(
                            ps_sq[:, 1, :],
                            KT[:, g, 1, t * 128:(t + 1) * 128],
                            QT[:, g, j * FB:(j + 1) * FB],
                            start=True, stop=True,
                        )
                        ex = pex.tile([128, 2, FB], BF16,
                                      tag="exp", name="exp")
                        nc.scalar.activation(
                            ex[:], ps_sq[:],
                            mybir.ActivationFunctionType.Exp,
                            scale=0.125,
                        )
                        # fused both-head mask multiply via 0-stride
                        # broadcast of the mask along the head dim
                        msl = mask_j[:, t, :].rearrange(
                            "p (a f) -> p a f", a=1)
                        _, m_b = broadcast_tensor_aps(ex[:], msl)
                        nc.vector.tensor_mul(ex[:], ex[:], m_b)
                        exs[t] = ex
                    # PV for the PREVIOUS phase's pair (mexp all resident)
                    if prev_pair is not None and t < NT:
                        exp_t = prev_exs[t]
                        nc.tensor.matmul(
                            ps_ctx[:, 0, :],
                            Vsb[:, t, pne * HP1:(pne + 1) * HP1],
                            exp_t[:, 0, :],
                            start=(t == 0), stop=(t == NT - 1),
                        )
                        nc.tensor.matmul(
                            ps_ctx[:, 1, :],
                            Vsb[:, t, pno * HP1:(pno + 1) * HP1],
                            exp_t[:, 1, :],
                            start=(t == 0), stop=(t == NT - 1),
                        )
                # ---- phase epilogue for the pair whose PV just finished:
                # per-head evac + recip + f16 shadow; the bc-matmul +
                # norm-mul + store are deferred via norm_q
                if prev_pair is not None:
                    ctx_sb = pctx.tile([HP1, 2, FB], F32, tag="ctx_sb",
                                       name="ctx_sb")
                    for i in range(2):
                        nc.vector.tensor_copy(ctx_sb[:, i, :], ps_ctx[:, i, :])
                        nc.vector.reciprocal_approx_fast(ctx_sb[0:1, i, :],
                                                         ctx_sb[0:1, i, :])
                        nc.vector.tensor_copy(rsh[0:1, pg, i, :],
                                              ctx_sb[0:1, i, :])
                    _norm_pair(pj, pg, ctx_sb)
                while mul_q:
                    mul_q.pop(0)()
                prev_pair, prev_exs = pair, exs
            while norm_q:
                bc, mul = norm_q.pop(0)
                bc()
                mul()

    nc.compile()
    return nc


_compiled = None


def _get_compiled():
    global _compiled
    if _compiled is None:
        _compiled = _program()
    return _compiled


def make_in_maps(from_tensor, to_tensor, attention_mask, wq, bq, wk, bk, wv, bv):
    bf = ml_dtypes.bfloat16
    from_tensor = np.asarray(from_tensor, dtype=np.float32)
    to_tensor = np.asarray(to_tensor, dtype=np.float32)
    attention_mask = np.asarray(attention_mask)
    wq = np.asarray(wq, dtype=np.float32)
    wk = np.asarray(wk, dtype=np.float32)
    wv = np.asarray(wv, dtype=np.float32)
    bq = np.asarray(bq, dtype=np.float32)
    bk = np.asarray(bk, dtype=np.float32)
    bv = np.asarray(bv, dtype=np.float32)

    fromT_b = [np.ascontiguousarray(from_tensor[b].T).astype(bf) for b in range(B)]
    toT_b = [np.ascontiguousarray(to_tensor[b].T).astype(bf) for b in range(B)]
    maskT_b = [attention_mask[b].T.astype(bf) for b in range(B)]
    vones_arr = np.zeros((128, 128), dtype=bf)
    vones_arr[0, :] = 1.0
    sel64_arr = np.zeros((128, 128), dtype=np.float16)
    sel64_arr[0, :] = 1.0

    in_maps = []
    for c in range(NCORES):
        b, hb = divmod(c, NCORES // B)
        hs = hb * HPC
        bq_dev = bq[hs:hs + HPC].reshape(NG, 128).T
        bk_dev = bk[hs:hs + HPC].reshape(NG, 128).T
        bv_pad = np.zeros((128, HPC * H), dtype=bf)
        bv_pad[0, :] = bv[hs:hs + HPC].reshape(HPC * H)
        in_maps.append(
            dict(
                fromT=fromT_b[b],
                toT=toT_b[b],
                maskT=maskT_b[b],
                wq=wq[:, hs:hs + HPC, :].reshape(D, HPC * H).astype(bf),
                wk=wk[:, hs:hs + HPC, :].reshape(D, HPC * H).astype(bf),
                wv=wv[:, hs:hs + HPC, :].reshape(D, HPC * H).astype(bf),
                bqk=np.ascontiguousarray(
                    np.concatenate([bq_dev, bk_dev], axis=1), dtype=np.float32
                ),
                bv_pad=bv_pad,
                vones=vones_arr,
                sel64=sel64_arr,
            )
        )
    return in_maps


def gather_output(results):
    out = np.empty((B, F, N, H), dtype=np.float32)
    for c in range(NCORES):
        b, hb = divmod(c, NCORES // B)
        hs = hb * HPC
        ctx = results[c]["out_ctx"]  # [HPC, H, F]
        out[b, :, hs:hs + HPC, :] = ctx.transpose(2, 0, 1)
    return out


def run_sharded(inputs, **run_kwargs):
    """Run the SPMD kernel; returns (output, BassKernelResults)."""
    nc = _get_compiled()
    in_maps = make_in_maps(**inputs)
    res = run_bass_kernel_spmd(nc, in_maps, list(range(NCORES)), **run_kwargs)
    return gather_output(res.results), res


def kernel(**inputs):
    out, _ = run_sharded(inputs)
    return out
